# revision 1
# baseline (speedup 1.0000x reference)
"""Trainium2 Bass kernel for a VAE-style AttnBlock.

Reference semantics (B=4, C=512, H=W=64, HW=4096):
    h   = GroupNorm32(x)                                  (fp32 stats)
    q/k/v = 1x1 conv(h)                                   (C x C weights)
    attn  = softmax(q^T k / sqrt(C)) over keys            (HW x HW per sample)
    out   = attn @ v
    y     = x + 1x1 conv(out)

Sharding: 8 cores = 4 samples x 2 query-halves. Each core gets its
sample's full x (spatially rotated so its query half sits in columns
[0:2048]) and computes GroupNorm + full K/V redundantly, queries /
attention for its 2048 columns. Attention is permutation-equivariant
over spatial positions, so the rotation is exact.

Key restructurings vs the fp32 reference:
  * wo is folded into wv on the host (W' = wo @ wv), so attn @ v'
    IS the output projection; the PV accumulators only need softmax
    normalization (divide by den) plus the residual. boeff = wo@bv+bo.
  * Everything on the PE runs fp8e4m3 DoubleRow (Q/K/V' projections,
    S^T = k^T q, PV). fp8 weights are prescaled by 8 on the host to
    clear the e4m3 subnormal band; the scales cancel in the exp logit
    scale (1/(64 sqrt(C))) and in den (ones_m = 8).
  * x is cast to bf16 on the host: halves the input DMA and lets x
    stay resident in SBUF for the residual add.
  * GroupNorm stats use bn_stats/bn_aggr per 512-column chunk,
    finalized per channel-tile as the tile-major x DMA lands, so the
    normalize-apply (written directly to fp8 h8) and the Q/K/V'
    projections start as soon as the last tile arrives.
  * S^T is computed transposed (k^T q) so PV needs no transposes;
    softmax skips the max (logits ~N(0,1)); exp is shifted by -3 so P
    fits fp8 (cancels in P/den). den rides a ones-matmul accumulated
    with PV. 1/den uses the single-op reciprocal_approx_fast; the
    residual add is a fused scalar_tensor_tensor. y returns as bf16.

Schedule notes (from NTFF profiles): dummy matmuls keep the PE's HAM
clock-gate warm through the DMA/stats window; each query-chunk's tail
is interleaved one-action-per-S^T-step into the next chunk's stream so
the PE never idles.
"""

import sys

for _p in ("/opt/trn_rl_repo",):
    if _p not in sys.path:
        sys.path.insert(0, _p)

import numpy as np
import ml_dtypes

C = 512
HW = 4096
NQ = 2048  # queries per core
CT = 4  # channel tiles of 128
MT = 32  # m (key) tiles of 128
NCHUNK = 512  # free-dim chunk (one PSUM bank of fp32)
NUM_GROUPS = 32
GSIZE = C // NUM_GROUPS  # 16 channels per group
EPS = 1e-6
N_CORES = 8
WARMUP_MMS = 28  # upfront dummy matmuls covering init until the x DMA lands
PV_LAG = 4  # S^T tile-pairs emitted ahead of their PV consumers
W_SCALE = 8.0  # fp8 weights are prescaled by this (cleared subnormals)
EXP_SHIFT = 3.0  # exp(s - shift): keeps P inside fp8e4m3 range; cancels in P/den

_compiled = None


def _build_program():
    import concourse.bacc as bacc
    import concourse.mybir as mybir
    import concourse.tile as tile

    f32 = mybir.dt.float32
    bf16 = mybir.dt.bfloat16
    fp8 = mybir.dt.float8e4
    ALU = mybir.AluOpType
    ACT = mybir.ActivationFunctionType
    DR = mybir.MatmulPerfMode.DoubleRow

    nc = bacc.Bacc("TRN2", target_bir_lowering=False, debug=False, num_devices=N_CORES)

    x_d = nc.dram_tensor("x", [C, HW], bf16, kind="ExternalInput").ap()
    w_d = {
        name: nc.dram_tensor(name, [C, C], fp8, kind="ExternalInput").ap()
        for name in ("wqT8", "wkT8", "wvoT8")
    }
    vec_d = {
        name: nc.dram_tensor(name, [C], f32, kind="ExternalInput").ap()
        for name in ("bq", "bk", "boeff", "gamma", "beta")
    }
    gind_d = nc.dram_tensor("gind", [128, 128], f32, kind="ExternalInput").ap()
    y_d = nc.dram_tensor("y", [C, NQ], bf16, kind="ExternalOutput").ap()

    xr = x_d.rearrange("(t p) m -> p t m", p=128)
    yr = y_d.rearrange("(t p) n -> p t n", p=128)

    # q_sb/k_sb hold 8*(w h + b); the exp scale removes the 64x.
    scale_exp = float(C) ** -0.5 / (W_SCALE * W_SCALE)
    NJ = HW // NCHUNK  # 8 chunks over keys
    NJQ = NQ // NCHUNK  # 4 chunks over queries
    BNS = 6  # bn_stats output slots

    with tile.TileContext(nc) as tc:
        with (
            tc.tile_pool(name="consts", bufs=1) as cp,
            tc.tile_pool(name="big", bufs=1) as bp,
        ):
            # tiles only; DMA issues are ordered inside phase A across two
            # engine queues (each dma_start costs ~0.64us of issue time)
            w_sb = {
                name: cp.tile([128, CT, C], fp8, name=f"{name}_sb")
                for name in ("wqT8", "wkT8", "wvoT8")
            }

            x_sb = bp.tile([128, CT, HW], bf16, name="x_sb")
            h8 = bp.tile([128, CT, HW], fp8, name="h8")
            q_sb = bp.tile([128, CT, NQ], fp8, name="q_sb")
            k_sb = bp.tile([128, CT, HW], fp8, name="k_sb")
            vT_sb = bp.tile([128, MT, NCHUNK], fp8, name="vT_sb")

            v_sb = {
                name: cp.tile([128, CT], f32, name=f"{name}_sb")
                for name in ("bq", "bk", "boeff", "gamma", "beta")
            }
            gind_sb = cp.tile([128, 128], f32, name="gind_sb")
            # W_SCALE-valued "ones": den = 8 * sum(P) absorbs the 8x in v'.
            ones_m = cp.tile([128, 2, 128], fp8, name="ones_m")
            nc.vector.memset(ones_m[:], W_SCALE)
            eps_sb = cp.tile([128, 1], f32, name="eps_sb")
            nc.vector.memset(eps_sb[:], EPS)
            shift_sb = cp.tile([128, 1], f32, name="shift_sb")
            nc.vector.memset(shift_sb[:], -EXP_SHIFT)
            stats = cp.tile([128, CT, 2], f32, name="stats")  # scale, shift

            # ---------------- GroupNorm + QKV (tile-pipelined) ----------------
            # x arrives tile-major; each channel-tile's stats (bn_stats per
            # chunk, bn_aggr + group-combine via the gind matmul) finalize as
            # soon as its 8 chunks land, and its normalize-apply (fp8 h8,
            # fused scale/shift) runs during the next tile's DMA. Only tile
            # 3's applies gate the projection loop.
            with (
                tc.tile_pool(name="gnwork", bufs=1) as gw,
                tc.tile_pool(name="gnps", bufs=1, space="PSUM") as gnps,
                tc.tile_pool(name="warmps", bufs=1, space="PSUM") as wps,
                tc.tile_pool(name="p2ps", bufs=6, space="PSUM") as p2,
            ):
                # x goes as 16 1024-column chunks (2KB rows). Scalar issues
                # tiles 0-1 plus gamma/beta; Sync issues wk/gind first (needed
                # by the first finalize), tiles 2-3, then the remaining
                # weights and biases (first used ~20us later).
                WCH = 2 * NCHUNK
                for t in range(2):
                    for dd in range(NJ // 2):
                        ds = slice(dd * WCH, (dd + 1) * WCH)
                        nc.scalar.dma_start(x_sb[:, t, ds], xr[:, t, ds])
                nc.sync.dma_start(
                    w_sb["wkT8"][:], w_d["wkT8"].rearrange("(t p) o -> p t o", p=128)
                )
                nc.sync.dma_start(gind_sb[:], gind_d[:])
                for name in ("gamma", "beta"):
                    nc.sync.dma_start(
                        v_sb[name][:], vec_d[name].rearrange("(t p) -> p t", p=128)
                    )
                for t in range(2, CT):
                    for dd in range(NJ // 2):
                        ds = slice(dd * WCH, (dd + 1) * WCH)
                        nc.sync.dma_start(x_sb[:, t, ds], xr[:, t, ds])
                for name in ("wqT8", "wvoT8"):
                    nc.sync.dma_start(
                        w_sb[name][:], w_d[name].rearrange("(t p) o -> p t o", p=128)
                    )
                for name in ("bq", "bk", "boeff"):
                    nc.sync.dma_start(
                        v_sb[name][:], vec_d[name].rearrange("(t p) -> p t", p=128)
                    )

                # PE warm-up: keeps the HAM activity window busy while the
                # DVE runs bn_stats. An upfront burst on a memset tile covers
                # init; the rest are paced by the x DMA chunks.
                warm = wps.tile([128, NCHUNK], f32, name="warm")
                wtmp = cp.tile([128, NCHUNK], bf16, name="wtmp")
                nc.gpsimd.memset(wtmp[:], 0.5)
                for _ in range(WARMUP_MMS):
                    nc.tensor.matmul(
                        warm[:], wtmp[:, 0:128], wtmp[:], start=True, stop=True
                    )

                bns = gw.tile([128, CT, NJ, BNS], f32, name="bns")
                mv = gw.tile([128, CT, 2], f32, name="mv")
                sums = gw.tile([128, CT, 2], f32, name="sums")
                sv = gw.tile([128, CT, 4], f32, name="sv")  # Mg, Eg2, -Mg, -var
                sd = gw.tile([128, CT, 2], f32, name="sd")  # sqrt, rstd
                gps_all = gnps.tile([128, CT, 2], f32, name="gps_all")

                def finalize_tile(t):
                    # per-partition (mean, var) over the full row
                    nc.vector.bn_aggr(mv[:, t, :], bns[:, t, :, :])
                    # sums = (mean_p, E_p[x^2]) for the group matmul
                    nc.scalar.copy(sums[:, t, 0:1], mv[:, t, 0:1])
                    nc.vector.scalar_tensor_tensor(
                        sums[:, t, 1:2], mv[:, t, 0:1], mv[:, t, 0:1],
                        mv[:, t, 1:2], ALU.mult, ALU.add,
                    )
                    # broadcast group sums to every member partition
                    nc.tensor.matmul(
                        gps_all[:, t, :], gind_sb[:], sums[:, t, :],
                        start=True, stop=True,
                    )
                    nc.vector.tensor_scalar(
                        sv[:, t, 0:2], gps_all[:, t, :], 1.0 / GSIZE, None, ALU.mult
                    )
                    nc.vector.tensor_scalar(
                        sv[:, t, 2:3], gps_all[:, t, 0:1], -1.0 / GSIZE, None, ALU.mult
                    )
                    # -var = Mg^2 - Eg2; Sqrt(-1 * -var + eps) = sqrt(var+eps)
                    nc.vector.scalar_tensor_tensor(
                        sv[:, t, 3:4], sv[:, t, 0:1], sv[:, t, 0:1],
                        sv[:, t, 1:2], ALU.mult, ALU.subtract,
                    )
                    nc.scalar.activation(
                        sd[:, t, 0:1], sv[:, t, 3:4], ACT.Sqrt,
                        bias=eps_sb[:], scale=-1.0,
                    )
                    nc.vector.reciprocal(sd[:, t, 1:2], sd[:, t, 0:1])
                    nc.vector.tensor_tensor(
                        stats[:, t, 0:1], sd[:, t, 1:2], v_sb["gamma"][:, t : t + 1],
                        ALU.mult,
                    )
                    # shift = beta - Mg*scale
                    nc.vector.scalar_tensor_tensor(
                        stats[:, t, 1:2], sv[:, t, 2:3], stats[:, t, 0:1],
                        v_sb["beta"][:, t : t + 1], ALU.mult, ALU.add,
                    )

                def apply_chunk(t, jj):
                    # DVE carries bn_stats + the k copies, so most applies go
                    # to ACT (measured: DVE 73us vs ACT 51us pre-attention)
                    js = slice(jj * NCHUNK, (jj + 1) * NCHUNK)
                    if (jj * CT + t) % 3 != 0:
                        nc.scalar.activation(
                            h8[:, t, js], x_sb[:, t, js], ACT.Identity,
                            bias=stats[:, t, 1:2], scale=stats[:, t, 0:1],
                        )
                    else:
                        nc.vector.tensor_scalar(
                            h8[:, t, js], x_sb[:, t, js],
                            stats[:, t, 0:1], stats[:, t, 1:2],
                            ALU.mult, ALU.add,
                        )

                # bf16 pacer column rewritten from the stats stream: filler
                # matmuls with it as weights track the stats tail so the PE
                # busy-run doesn't break at the stats->projection junction.
                pacer = cp.tile([128, 1], bf16, name="pacer")

                def filler(src):
                    nc.scalar.copy(pacer[:], src)
                    nc.tensor.matmul(
                        warm[0:1, :], pacer[:], wtmp[:], start=True, stop=True
                    )

                # phase A keeps the DVE stream pure bn_stats so tile 3's
                # stats land as early as possible; all applies run inside the
                # projection loop below.
                for t in range(CT):
                    for jj in range(NJ):
                        js = slice(jj * NCHUNK, (jj + 1) * NCHUNK)
                        nc.vector.bn_stats(bns[:, t, jj, :], x_sb[:, t, js])
                        if jj % 2 == 0:
                            # paced warm-up: depends on this DMA chunk, so the
                            # PE shows activity at the pace x actually arrives
                            nc.tensor.matmul(
                                warm[0:1, 0:256],
                                wtmp[:, 0:1],
                                x_sb[:, t, jj * NCHUNK : jj * NCHUNK + 256],
                                start=True, stop=True,
                            )
                    finalize_tile(t)
                    filler(sd[:, t, 1:2])
                    filler(stats[:, t, 1:2])

                # -- projection loop; tile 3's applies interleave per chunk --
                def dr_proj(ps, w, o):
                    for T in range(CT // 2):
                        nc.tensor.matmul(
                            ps[:],
                            w[:, 2 * T : 2 * T + 2, o * 128 : (o + 1) * 128],
                            h8[:, 2 * T : 2 * T + 2, js],
                            start=(T == 0),
                            stop=(T == CT // 2 - 1),
                            perf_mode=DR,
                        )

                for jj in range(NJ):
                    js = slice(jj * NCHUNK, (jj + 1) * NCHUNK)
                    for t in range(CT):
                        apply_chunk(t, jj)
                    # apply-paced filler: bridges any stall while the chunk's
                    # h8 lands so the PE clock stays ramped
                    nc.tensor.matmul(
                        warm[0:1, :],
                        h8[:, 3, jj * NCHUNK : jj * NCHUNK + 1],
                        h8[:, 3, js],
                        start=True, stop=True,
                    )
                    for o in range(CT):
                        ps = p2.tile([128, NCHUNK], f32, name="psk", tag="p2")
                        dr_proj(ps, w_sb["wkT8"], o)
                        nc.vector.tensor_scalar(
                            k_sb[:, o, js], ps[:],
                            v_sb["bk"][:, o : o + 1], None, ALU.add,
                        )
                    if jj < NJQ:
                        for o in range(CT):
                            ps = p2.tile([128, NCHUNK], f32, name="psq", tag="p2")
                            dr_proj(ps, w_sb["wqT8"], o)
                            nc.scalar.activation(
                                q_sb[:, o, js], ps[:], ACT.Identity,
                                bias=v_sb["bq"][:, o : o + 1],
                            )
                    for i, u in enumerate(range(4 * jj, 4 * jj + 4)):
                        ps = p2.tile([128, NCHUNK], f32, name="psv", tag="p2")
                        for T in range(CT // 2):
                            nc.tensor.matmul(
                                ps[:],
                                h8[:, 2 * T : 2 * T + 2, u * 128 : (u + 1) * 128],
                                w_sb["wvoT8"][:, 2 * T : 2 * T + 2, :],
                                start=(T == 0),
                                stop=(T == CT // 2 - 1),
                                perf_mode=DR,
                            )
                        # engine balance: one v' copy per chunk on DVE
                        if i == 0:
                            nc.vector.tensor_copy(vT_sb[:, u, :], ps[:])
                        else:
                            nc.scalar.copy(vT_sb[:, u, :], ps[:])

            # ------- attention (tail-overlapped; PV is the projection) ----
            with (
                tc.tile_pool(name="sps", bufs=3, space="PSUM") as sp,
                tc.tile_pool(name="pvps", bufs=1, space="PSUM") as pvp,
                tc.tile_pool(name="w3", bufs=2) as w3,
                tc.tile_pool(name="ptp", bufs=7) as ptp,
                tc.tile_pool(name="iop", bufs=2) as iop,
            ):
                state = {}  # per-j: pv, den, pts, y_sb

                def alloc_pv(j):
                    state[j]["pv"] = [
                        pvp.tile([128, NCHUNK], f32, name=f"pv{o}", tag=f"pv{o}")
                        for o in range(CT)
                    ]
                    state[j]["den"] = pvp.tile([128, NCHUNK], f32, name="den", tag="den")

                def s_tile(j, u):
                    njs = slice(j * NCHUNK, (j + 1) * NCHUNK)
                    ssp = sp.tile([128, NCHUNK], f32, name="ssp", tag="s3")
                    for T in range(CT // 2):
                        nc.tensor.matmul(
                            ssp[:],
                            k_sb[:, 2 * T : 2 * T + 2, u * 128 : (u + 1) * 128],
                            q_sb[:, 2 * T : 2 * T + 2, njs],
                            start=(T == 0),
                            stop=(T == CT // 2 - 1),
                            perf_mode=DR,
                        )
                    if u % 2 == 0:
                        pt = ptp.tile([128, 2, NCHUNK], fp8, name="pt", tag="pt")
                        state[j]["pts"][u // 2] = pt
                    nc.scalar.activation(
                        state[j]["pts"][u // 2][:, u % 2, :], ssp[:],
                        ACT.Exp, scale=scale_exp, bias=shift_sb[:],
                    )

                NPAIR = MT // 2

                def emit_pv(j, uu):
                    stj = state[j]
                    nc.tensor.matmul(
                        stj["den"][:], ones_m[:], stj["pts"][uu][:],
                        start=(uu == 0), stop=(uu == NPAIR - 1), perf_mode=DR,
                    )
                    for o in range(CT):
                        nc.tensor.matmul(
                            stj["pv"][o][:],
                            vT_sb[:, 2 * uu : 2 * uu + 2, o * 128 : (o + 1) * 128],
                            stj["pts"][uu][:],
                            start=(uu == 0), stop=(uu == NPAIR - 1), perf_mode=DR,
                        )
                    stj["pts"][uu] = None

                # Tail work for chunk j, split into small actions interleaved
                # one-per-S^T-step into the next chunk's stream. PV already
                # carries the output projection, so the tail is just
                # normalize (x recip) + residual (fused) + DMA.
                actions = []

                def tail_start(j):
                    stj = state.pop(j)
                    njs = slice(j * NCHUNK, (j + 1) * NCHUNK)
                    y_sb = iop.tile([128, CT, NCHUNK], bf16, name="y_sb", tag="y")
                    recipb = w3.tile([128, NCHUNK], f32, name="recipb", tag="recipb")

                    def recip_step():
                        nc.vector.reciprocal_approx_fast(recipb[:], stj["den"][:])

                    def norm_step(o):
                        def go():
                            tsb = w3.tile([128, NCHUNK], f32, name="tsb", tag="tsb")
                            nc.vector.tensor_tensor(
                                tsb[:], stj["pv"][o][:], recipb[:], ALU.mult
                            )
                            nc.vector.scalar_tensor_tensor(
                                y_sb[:, o, :], x_sb[:, o, njs],
                                v_sb["boeff"][:, o : o + 1], tsb[:],
                                ALU.add, ALU.add,
                            )
                            # per-o DMA: the last chunk's writeback overlaps
                            # the remaining normalize steps
                            nc.sync.dma_start(yr[:, o, njs], y_sb[:, o, :])
                        return go

                    actions.append(recip_step)
                    for o in range(CT):
                        actions.append(norm_step(o))

                pending = []

                def pop_one():
                    jj, pp = pending.pop(0)
                    if pp == 0:
                        alloc_pv(jj)
                    emit_pv(jj, pp)
                    if pp == NPAIR - 1:
                        tail_start(jj)

                for j in range(NJQ):
                    state[j] = {"pts": [None] * NPAIR}
                    for u in range(MT):
                        s_tile(j, u)
                        if u % 2 == 1:
                            pending.append((j, u // 2))
                            if len(pending) > PV_LAG:
                                pop_one()
                        if actions:
                            actions.pop(0)()
                while pending:
                    pop_one()
                while actions:
                    actions.pop(0)()

    nc.compile()
    return nc


def get_program():
    global _compiled
    if _compiled is None:
        _compiled = _build_program()
    return _compiled


def make_in_maps(x, gn_gamma, gn_beta, wq, bq, wk, bk, wv, bv, wo, bo):
    bf = ml_dtypes.bfloat16
    f8 = ml_dtypes.float8_e4m3
    wvo = (wv.astype(np.float64).T @ wo.astype(np.float64).T).astype(np.float32)
    shared = {
        "wqT8": np.ascontiguousarray(wq.T * W_SCALE).astype(f8),
        "wkT8": np.ascontiguousarray(wk.T * W_SCALE).astype(f8),
        "wvoT8": np.ascontiguousarray(wvo * W_SCALE).astype(f8),
        "bq": np.ascontiguousarray(bq * W_SCALE, np.float32).astype(np.float32),
        "bk": np.ascontiguousarray(bk * W_SCALE, np.float32).astype(np.float32),
        "boeff": (wo.astype(np.float64) @ bv.astype(np.float64) + bo).astype(np.float32),
        "gamma": np.ascontiguousarray(gn_gamma, np.float32),
        "beta": np.ascontiguousarray(gn_beta, np.float32),
        "gind": (np.arange(128)[:, None] // GSIZE == np.arange(128)[None, :] // GSIZE
                 ).astype(np.float32),
    }
    in_maps = []
    for core in range(N_CORES):
        b, half = core // 2, core % 2
        xs = np.asarray(x[b], np.float32).reshape(C, HW)
        if half:
            xs = np.concatenate([xs[:, NQ:], xs[:, :NQ]], axis=1)
        in_maps.append({"x": np.ascontiguousarray(xs.astype(bf)), **shared})
    return in_maps


def assemble_output(results, B, Hdim, Wdim):
    y = np.empty((B, C, HW), np.float32)
    for core in range(N_CORES):
        b, half = core // 2, core % 2
        y[b, :, half * NQ : (half + 1) * NQ] = results[core]["y"].astype(np.float32)
    return y.reshape(B, C, Hdim, Wdim)


def kernel(**inputs):
    from concourse.bass_utils import run_bass_kernel_spmd

    x = np.asarray(inputs["x"])
    B, _, Hdim, Wdim = x.shape
    nc = get_program()
    in_maps = make_in_maps(**inputs)
    res = run_bass_kernel_spmd(nc, in_maps, core_ids=list(range(N_CORES)))
    return assemble_output(res.results, B, Hdim, Wdim)


if __name__ == "__main__":
    rng = np.random.default_rng(0)
    ins = {
        "x": rng.standard_normal((4, C, 64, 64), np.float32),
        "gn_gamma": np.ones(C, np.float32),
        "gn_beta": np.zeros(C, np.float32),
    }
    s = 1.0 / np.sqrt(C)
    for nm in ("q", "k", "v", "o"):
        ins[f"w{nm}"] = rng.standard_normal((C, C), np.float32).astype(np.float32) * s
        ins[f"b{nm}"] = np.zeros(C, np.float32)
    out = kernel(**ins)
    print("kernel ran, out shape", out.shape, out.dtype)

